# revision 42
# baseline (speedup 1.0000x reference)
"""Bahdanau-attention kernel for Trainium2, data-parallel over batch on 8 NeuronCores.

reference math (per batch row b):
    h_proj = hidden @ Wh.T + attn_b                       # [256]
    e_proj = enc_out[b] @ We.T                            # [2048, 256]
    energy = tanh(h_proj + e_proj)                        # [2048, 256]
    scores = energy @ v + v_b                             # [2048]   (v_b drops out of softmax)
    weights = softmax(scores)                             # [2048]
    context = weights @ enc_out[b]                        # [512]
returns (context [64, 512], weights [64, 2048])

Device strategy (per core, 8 batch rows):
  - host pre-transposes enc to encT[b] = enc[b].T (bf16); each row is loaded by
    four column-sliced DMAs so the first matmul chunk can start ~1.5us in.
  - e_projT[d, n] accumulated over 4 e-tiles with WeT stationary; tanh + h_proj
    bias fused on ScalarE over [128, 1024] chunks (PSUM -> SBUF bf16).
  - scores use a column-replicated v as stationary, so the score row lands
    replicated on all 128 PSUM partitions: softmax runs full-width with zero
    cross-partition traffic and 1/Z is already broadcast for the context.
  - softmax without max-subtraction (|scores| <= 16, exp is safe in fp32);
    exp + partial sum fused on ScalarE per 1024-half, 1/Z on VectorE.
  - context[e] = sum_n (encT[e, n] * rz) * p[n] on VectorE scalar_tensor_tensor
    (the 1/Z folded in as the per-partition scalar; out is a stride-0 dummy).
  - input loads ride the Sync HWDGE queue; constants go via GpSimd SWDGE, and
    results accumulate in persistent tiles that leave as two contiguous DMAs at
    the end (strided or fresh-data-gated output DMAs stall the pipeline).
  - a 21-matmul warm-up chain on zeroed SBUF trips the PE HAM clock gate to
    8/8 while the first enc DMAs are in flight.
"""

import numpy as np
import ml_dtypes

ENC_DIM, DEC_DIM = 512, 256
B, N = 64, 2048
N_CORES = 8
B_LOC = B // N_CORES  # 8
E_TILES = ENC_DIM // 128  # 4
D_TILES = DEC_DIM // 128  # 2
NC = 512  # n-chunk for DMA slices / scores matmuls

_BF16 = ml_dtypes.bfloat16
_CACHE = {}


def _build():
    import concourse.bass as bass  # noqa: F401
    import concourse.tile as tile
    from concourse import bacc, mybir

    f32 = mybir.dt.float32
    bf16 = mybir.dt.bfloat16

    nc = bacc.Bacc("TRN2", target_bir_lowering=False, debug=False, num_devices=N_CORES)

    encT = nc.dram_tensor("encT", [B_LOC, ENC_DIM, N], bf16, kind="ExternalInput")
    hb = nc.dram_tensor("hb", [D_TILES, 128, B_LOC], f32, kind="ExternalInput")
    wet = nc.dram_tensor("wet", [ENC_DIM, DEC_DIM], bf16, kind="ExternalInput")
    vrep = nc.dram_tensor("vrep", [D_TILES, 128, 128], bf16, kind="ExternalInput")
    ctx_out = nc.dram_tensor("ctx_out", [128, B_LOC * E_TILES], f32, kind="ExternalOutput")
    w_out = nc.dram_tensor("w_out", [1, B_LOC * N], bf16, kind="ExternalOutput")
    warm_out = nc.dram_tensor("warm_out", [1, 1], bf16, kind="ExternalOutput")

    with tile.TileContext(nc) as tc:
        with (
            tc.tile_pool(name="const", bufs=1) as const_pool,
            tc.tile_pool(name="enc", bufs=8) as enc_pool,
            tc.tile_pool(name="energy", bufs=2) as energy_pool,
            tc.tile_pool(name="prep", bufs=2) as p_pool,
            tc.tile_pool(name="small", bufs=2) as small_pool,
            tc.tile_pool(name="pe_ps", bufs=2, space="PSUM") as pe_ps_pool,
            tc.tile_pool(name="sc_ps", bufs=2, space="PSUM") as sc_ps_pool,
        ):
            # row 0's first column slice goes out before everything else so
            # its e_proj can start the moment the warmup chain ends
            enc_b0 = enc_pool.tile(
                [128, E_TILES * N], bf16, name="enc_all", tag="enc_all"
            )
            nc.sync.dma_start(
                enc_b0[:].rearrange("p (j n) -> p j n", j=E_TILES)[:, :, 0:NC],
                encT.ap()[0, :, 0:NC].rearrange("(j p) n -> p j n", p=128),
            )

            wet_t = []
            for k in range(E_TILES):
                t = const_pool.tile([128, DEC_DIM], bf16, tag=f"wet{k}")
                nc.sync.dma_start(t[:], wet.ap()[k * 128 : (k + 1) * 128, :])
                wet_t.append(t)
            vrep_t = []
            hb_t = []
            for m in range(D_TILES):
                t = const_pool.tile([128, 128], bf16, tag=f"vrep{m}")
                nc.sync.dma_start(t[:], vrep.ap()[m])
                vrep_t.append(t)
                t = const_pool.tile([128, B_LOC], f32, tag=f"hb{m}")
                nc.sync.dma_start(t[:], hb.ap()[m])
                hb_t.append(t)

            ctx_all = const_pool.tile(
                [128, B_LOC * E_TILES], f32, tag="ctx_all"
            )
            w_all = const_pool.tile([1, B_LOC * N], bf16, tag="w_all")

            # PE warm-up: back-to-back matmuls on zeroed SBUF while the
            # first enc DMAs are in flight, so the HAM clock gate is at 8/8
            # before real work lands. Result is written to warm_out so DCE
            # can't drop the chain.
            warm_sb = const_pool.tile([128, 640], bf16, tag="warm_sb")
            nc.vector.memset(warm_sb[:], 0)
            warm_ps = pe_ps_pool.tile([128, 1024], mybir.dt.float32, tag="pe")
            for w in range(21):
                nc.tensor.matmul(
                    warm_ps[:, 0:512],
                    warm_sb[:, 0:128],
                    warm_sb[:, 128:640],
                    start=(w == 0),
                    stop=(w == 20),
                )
            nc.scalar.activation(
                warm_sb[0:1, 0:1],
                warm_ps[0:1, 0:1],
                mybir.ActivationFunctionType.Identity,
            )
            nc.sync.dma_start(warm_out.ap(), warm_sb[0:1, 0:1])
            for b in range(B_LOC):
                # enc row [128, j*N + n]; four column-slice DMAs so chunk 0 is
                # usable before the whole 2 MB row has landed.
                if b == 0:
                    enc_all = enc_b0
                else:
                    enc_all = enc_pool.tile(
                        [128, E_TILES * N], bf16, name="enc_all", tag="enc_all"
                    )
                enc_v = enc_all[:].rearrange("p (j n) -> p j n", j=E_TILES)
                for c in range(1, 4) if b == 0 else range(4):
                    ns = slice(c * NC, (c + 1) * NC)
                    # first rows: use the idle ScalarE HWDGE ring for half the
                    # slices so row 0 lands in ~3us instead of ~6
                    eng = nc.scalar if (b < 2 and c % 2 == 1) else nc.sync
                    eng.dma_start(
                        enc_v[:, :, ns],
                        encT.ap()[b, :, ns].rearrange("(j p) n -> p j n", p=128),
                    )

                energy_t = [
                    energy_pool.tile(
                        [128, N], bf16, name=f"energy{m}", tag=f"energy{m}"
                    )
                    for m in range(D_TILES)
                ]
                p_rep = p_pool.tile([128, N], bf16)
                zc = small_pool.tile([128, 2], f32, tag="zc")

                def eproj_group(c2, m):
                    pe = pe_ps_pool.tile(
                        [128, 1024], mybir.dt.float32, name="pe", tag="pe"
                    )
                    for h in range(2):
                        ns = c2 * 1024 + h * NC
                        for k in range(E_TILES):
                            nc.tensor.matmul(
                                pe[:, h * NC : (h + 1) * NC],
                                wet_t[k][:, m * 128 : (m + 1) * 128],
                                enc_all[:, k * N + ns : k * N + ns + NC],
                                start=(k == 0),
                                stop=(k == E_TILES - 1),
                            )
                    nc.scalar.activation(
                        energy_t[m][:, c2 * 1024 : (c2 + 1) * 1024],
                        pe[:],
                        mybir.ActivationFunctionType.Tanh,
                        bias=hb_t[m][:, b : b + 1],
                    )

                def scores_group(c2):
                    ps = sc_ps_pool.tile(
                        [128, 1024], mybir.dt.float32, name="ps", tag="ps"
                    )
                    for h in range(2):
                        cs = slice(c2 * 1024 + h * NC, c2 * 1024 + (h + 1) * NC)
                        for m in range(D_TILES):
                            nc.tensor.matmul(
                                ps[:, h * NC : (h + 1) * NC],
                                vrep_t[m][:],
                                energy_t[m][:, cs],
                                start=(m == 0),
                                stop=(m == D_TILES - 1),
                            )
                    nc.scalar.activation(
                        p_rep[:, c2 * 1024 : (c2 + 1) * 1024],
                        ps[:],
                        mybir.ActivationFunctionType.Exp,
                        accum_out=zc[:, c2 : c2 + 1],
                    )

                # PE order: scores for half 0 slot in before the last e_proj
                # group so the tanh of each half has PE-time to hide under.
                eproj_group(0, 0)
                eproj_group(0, 1)
                eproj_group(1, 0)
                scores_group(0)
                eproj_group(1, 1)
                scores_group(1)

                # z = zc0 + zc1 on ScalarE (Identity + bias AP) to keep the
                # saturated VectorE free; fast-approx reciprocal is plenty
                # accurate next to bf16 data.
                z = small_pool.tile([128, 1], f32, tag="z")
                rz = small_pool.tile([128, 1], f32, tag="rz")
                with tc.high_priority():
                    nc.scalar.activation(
                        z[:],
                        zc[:, 0:1],
                        mybir.ActivationFunctionType.Identity,
                        bias=zc[:, 1:2],
                    )
                    nc.vector.reciprocal_approx_fast(rz[:], z[:])

                if b == B_LOC - 1:
                    # tail: weights row first on ScalarE, multiplies first on
                    # VectorE, so the copy-accums overlap the remaining STTs.
                    nc.scalar.mul(
                        w_all[0:1, b * N : (b + 1) * N], p_rep[0:1, :], rz[0:1, :]
                    )
                    tail_scr = []
                    for j in (2, 3):
                        scr = small_pool.tile(
                            [128, N], bf16, name=f"scr{j}", tag=f"scr{j}"
                        )
                        nc.vector.tensor_mul(
                            scr[:], enc_all[:, j * N : (j + 1) * N], p_rep[:]
                        )
                        tail_scr.append(scr)
                    for j in (2, 3):
                        sdummy = small_pool.tile(
                            [128, 1], bf16, name="sdummy", tag="sdummy"
                        )
                        nc.scalar.activation(
                            sdummy.broadcast_to((128, N)),
                            tail_scr[j - 2][:],
                            mybir.ActivationFunctionType.Copy,
                            scale=rz[:],
                            accum_out=ctx_all[
                                :, b * E_TILES + j : b * E_TILES + j + 1
                            ],
                        )
                for j in range(E_TILES):
                    if b == B_LOC - 1 and j >= 2:
                        continue
                    dummy = small_pool.tile(
                        [128, 1], bf16, name="dummy", tag="dummy"
                    )
                    nc.vector.scalar_tensor_tensor(
                        out=dummy.broadcast_to((128, N)),
                        in0=enc_all[:, j * N : (j + 1) * N],
                        scalar=rz[:],
                        in1=p_rep[:],
                        op0=mybir.AluOpType.mult,
                        op1=mybir.AluOpType.mult,
                        accum_out=ctx_all[:, b * E_TILES + j : b * E_TILES + j + 1],
                    )

                if b < B_LOC - 1:
                    # weights row: normalize only the replicated row 0 on
                    # ScalarE, off the critical path.
                    nc.scalar.mul(
                        w_all[0:1, b * N : (b + 1) * N], p_rep[0:1, :], rz[0:1, :]
                    )

            # bulk outputs leave while b7 finishes; only the b7 slices remain
            nc.sync.dma_start(
                ctx_out.ap()[:, : 7 * E_TILES], ctx_all[:, : 7 * E_TILES]
            )
            nc.sync.dma_start(w_out.ap()[:, : 7 * N], w_all[:, : 7 * N])
            nc.sync.dma_start(
                ctx_out.ap()[:, 7 * E_TILES :], ctx_all[:, 7 * E_TILES :]
            )
            nc.sync.dma_start(w_out.ap()[:, 7 * N :], w_all[:, 7 * N :])

    nc.compile()
    return nc


def _get_nc():
    if "nc" not in _CACHE:
        _CACHE["nc"] = _build()
    return _CACHE["nc"]


def make_in_maps(hidden, enc_out, attn_w, attn_b, v_w):
    hidden = np.asarray(hidden, dtype=np.float32)
    enc_out = np.asarray(enc_out, dtype=np.float32)
    attn_w = np.asarray(attn_w, dtype=np.float32)
    attn_b = np.asarray(attn_b, dtype=np.float32)
    v_w = np.asarray(v_w, dtype=np.float32)

    Wh = attn_w[:, :DEC_DIM]
    We = attn_w[:, DEC_DIM:]
    h_proj = hidden @ Wh.T + attn_b  # [64, 256]

    wet_np = np.ascontiguousarray(We.T).astype(_BF16)  # [512, 256]
    vrep_np = np.ascontiguousarray(
        np.broadcast_to(v_w[0].reshape(D_TILES, 128, 1), (D_TILES, 128, 128))
    ).astype(_BF16)

    in_maps = []
    for c in range(N_CORES):
        sl = slice(c * B_LOC, (c + 1) * B_LOC)
        encT_np = np.ascontiguousarray(
            enc_out[sl].transpose(0, 2, 1)
        ).astype(_BF16)  # [8, 512, 2048]
        hb_np = np.ascontiguousarray(
            h_proj[sl].reshape(B_LOC, D_TILES, 128).transpose(1, 2, 0)
        )  # [2, 128, 8]
        in_maps.append(
            {"encT": encT_np, "hb": hb_np, "wet": wet_np, "vrep": vrep_np}
        )
    return in_maps


def kernel(hidden, enc_out, attn_w, attn_b, v_w, v_b):
    from concourse.bass_utils import run_bass_kernel_spmd

    nc = _get_nc()
    in_maps = make_in_maps(hidden, enc_out, attn_w, attn_b, v_w)
    res = run_bass_kernel_spmd(nc, in_maps, core_ids=list(range(N_CORES)))

    context = np.concatenate(
        [
            res.results[c]["ctx_out"]
            .reshape(128, B_LOC, E_TILES)
            .transpose(1, 2, 0)
            .reshape(B_LOC, ENC_DIM)
            for c in range(N_CORES)
        ],
        0,
    )
    weights = np.concatenate(
        [
            res.results[c]["w_out"].astype(np.float32).reshape(B_LOC, N)
            for c in range(N_CORES)
        ],
        0,
    )
    return (context, weights)


# revision 43
# speedup vs baseline: 1.0496x; 1.0496x over previous
"""Bahdanau-attention kernel for Trainium2, data-parallel over batch on 8 NeuronCores.

reference math (per batch row b):
    h_proj = hidden @ Wh.T + attn_b                       # [256]
    e_proj = enc_out[b] @ We.T                            # [2048, 256]
    energy = tanh(h_proj + e_proj)                        # [2048, 256]
    scores = energy @ v + v_b                             # [2048]   (v_b drops out of softmax)
    weights = softmax(scores)                             # [2048]
    context = weights @ enc_out[b]                        # [512]
returns (context [64, 512], weights [64, 2048])

Device strategy (per core, 8 batch rows):
  - host pre-transposes enc to encT[b] = enc[b].T (bf16); each row is loaded by
    four column-sliced DMAs so the first matmul chunk can start ~1.5us in.
  - e_projT[d, n] accumulated over 4 e-tiles with WeT stationary; tanh + h_proj
    bias fused on ScalarE over [128, 1024] chunks (PSUM -> SBUF bf16).
  - scores use a column-replicated v as stationary, so the score row lands
    replicated on all 128 PSUM partitions: softmax runs full-width with zero
    cross-partition traffic and 1/Z is already broadcast for the context.
  - softmax without max-subtraction (|scores| <= 16, exp is safe in fp32);
    exp + partial sum fused on ScalarE per 1024-half, 1/Z on VectorE.
  - context[e] = sum_n (encT[e, n] * rz) * p[n] on VectorE scalar_tensor_tensor
    (the 1/Z folded in as the per-partition scalar; out is a stride-0 dummy).
  - input loads ride the Sync HWDGE queue; constants go via GpSimd SWDGE, and
    results accumulate in persistent tiles that leave as two contiguous DMAs at
    the end (strided or fresh-data-gated output DMAs stall the pipeline).
  - a 21-matmul warm-up chain on zeroed SBUF trips the PE HAM clock gate to
    8/8 while the first enc DMAs are in flight.
"""

import numpy as np
import ml_dtypes

ENC_DIM, DEC_DIM = 512, 256
B, N = 64, 2048
N_CORES = 8
B_LOC = B // N_CORES  # 8
E_TILES = ENC_DIM // 128  # 4
D_TILES = DEC_DIM // 128  # 2
NC = 512  # n-chunk for DMA slices / scores matmuls

_BF16 = ml_dtypes.bfloat16
_CACHE = {}


def _build():
    import concourse.bass as bass  # noqa: F401
    import concourse.tile as tile
    from concourse import bacc, mybir

    f32 = mybir.dt.float32
    bf16 = mybir.dt.bfloat16

    nc = bacc.Bacc("TRN2", target_bir_lowering=False, debug=False, num_devices=N_CORES)

    encT = nc.dram_tensor("encT", [B_LOC, ENC_DIM, N], bf16, kind="ExternalInput")
    hb = nc.dram_tensor("hb", [D_TILES, 128, B_LOC], f32, kind="ExternalInput")
    wet = nc.dram_tensor("wet", [ENC_DIM, DEC_DIM], bf16, kind="ExternalInput")
    vrep = nc.dram_tensor("vrep", [D_TILES, 128, 128], bf16, kind="ExternalInput")
    ctx_out = nc.dram_tensor("ctx_out", [128, B_LOC * E_TILES], f32, kind="ExternalOutput")
    w_out = nc.dram_tensor("w_out", [1, B_LOC * N], bf16, kind="ExternalOutput")
    warm_out = nc.dram_tensor("warm_out", [1, 1], bf16, kind="ExternalOutput")

    with tile.TileContext(nc) as tc:
        with (
            tc.tile_pool(name="const", bufs=1) as const_pool,
            tc.tile_pool(name="enc", bufs=8) as enc_pool,
            tc.tile_pool(name="energy", bufs=2) as energy_pool,
            tc.tile_pool(name="prep", bufs=2) as p_pool,
            tc.tile_pool(name="small", bufs=2) as small_pool,
            tc.tile_pool(name="pe_ps", bufs=2, space="PSUM") as pe_ps_pool,
            tc.tile_pool(name="sc_ps", bufs=2, space="PSUM") as sc_ps_pool,
        ):
            wet_t = []
            for k in range(E_TILES):
                t = const_pool.tile([128, DEC_DIM], bf16, tag=f"wet{k}")
                nc.gpsimd.dma_start(t[:], wet.ap()[k * 128 : (k + 1) * 128, :])
                wet_t.append(t)
            vrep_t = []
            hb_t = []
            for m in range(D_TILES):
                t = const_pool.tile([128, 128], bf16, tag=f"vrep{m}")
                nc.gpsimd.dma_start(t[:], vrep.ap()[m])
                vrep_t.append(t)
                t = const_pool.tile([128, B_LOC], f32, tag=f"hb{m}")
                nc.gpsimd.dma_start(t[:], hb.ap()[m])
                hb_t.append(t)

            ctx_all = const_pool.tile(
                [128, B_LOC * E_TILES], f32, tag="ctx_all"
            )
            w_all = const_pool.tile([1, B_LOC * N], bf16, tag="w_all")

            # PE warm-up: back-to-back matmuls on zeroed SBUF while the
            # first enc DMAs are in flight, so the HAM clock gate is at 8/8
            # before real work lands. Result is written to warm_out so DCE
            # can't drop the chain.
            warm_sb = const_pool.tile([128, 640], bf16, tag="warm_sb")
            nc.vector.memset(warm_sb[:], 0)
            warm_ps = pe_ps_pool.tile([128, 1024], mybir.dt.float32, tag="pe")
            for w in range(21):
                nc.tensor.matmul(
                    warm_ps[:, 0:512],
                    warm_sb[:, 0:128],
                    warm_sb[:, 128:640],
                    start=(w == 0),
                    stop=(w == 20),
                )
            nc.scalar.activation(
                warm_sb[0:1, 0:1],
                warm_ps[0:1, 0:1],
                mybir.ActivationFunctionType.Identity,
            )
            nc.sync.dma_start(warm_out.ap(), warm_sb[0:1, 0:1])
            for b in range(B_LOC):
                # enc row [128, j*N + n]; four column-slice DMAs so chunk 0 is
                # usable before the whole 2 MB row has landed.
                enc_all = enc_pool.tile([128, E_TILES * N], bf16)
                enc_v = enc_all[:].rearrange("p (j n) -> p j n", j=E_TILES)
                for c in range(4):
                    ns = slice(c * NC, (c + 1) * NC)
                    # first rows: use the idle ScalarE HWDGE ring for half the
                    # slices so row 0 lands in ~3us instead of ~6
                    eng = nc.scalar if (b < 2 and c % 2 == 1) else nc.sync
                    eng.dma_start(
                        enc_v[:, :, ns],
                        encT.ap()[b, :, ns].rearrange("(j p) n -> p j n", p=128),
                    )

                energy_t = [
                    energy_pool.tile(
                        [128, N], bf16, name=f"energy{m}", tag=f"energy{m}"
                    )
                    for m in range(D_TILES)
                ]
                p_rep = p_pool.tile([128, N], bf16)
                zc = small_pool.tile([128, 2], f32, tag="zc")

                def eproj_group(c2, m):
                    pe = pe_ps_pool.tile(
                        [128, 1024], mybir.dt.float32, name="pe", tag="pe"
                    )
                    for h in range(2):
                        ns = c2 * 1024 + h * NC
                        for k in range(E_TILES):
                            nc.tensor.matmul(
                                pe[:, h * NC : (h + 1) * NC],
                                wet_t[k][:, m * 128 : (m + 1) * 128],
                                enc_all[:, k * N + ns : k * N + ns + NC],
                                start=(k == 0),
                                stop=(k == E_TILES - 1),
                            )
                    nc.scalar.activation(
                        energy_t[m][:, c2 * 1024 : (c2 + 1) * 1024],
                        pe[:],
                        mybir.ActivationFunctionType.Tanh,
                        bias=hb_t[m][:, b : b + 1],
                    )

                def scores_group(c2):
                    ps = sc_ps_pool.tile(
                        [128, 1024], mybir.dt.float32, name="ps", tag="ps"
                    )
                    for h in range(2):
                        cs = slice(c2 * 1024 + h * NC, c2 * 1024 + (h + 1) * NC)
                        for m in range(D_TILES):
                            nc.tensor.matmul(
                                ps[:, h * NC : (h + 1) * NC],
                                vrep_t[m][:],
                                energy_t[m][:, cs],
                                start=(m == 0),
                                stop=(m == D_TILES - 1),
                            )
                    nc.scalar.activation(
                        p_rep[:, c2 * 1024 : (c2 + 1) * 1024],
                        ps[:],
                        mybir.ActivationFunctionType.Exp,
                        accum_out=zc[:, c2 : c2 + 1],
                    )

                # PE order: scores for half 0 slot in before the last e_proj
                # group so the tanh of each half has PE-time to hide under.
                eproj_group(0, 0)
                eproj_group(0, 1)
                eproj_group(1, 0)
                scores_group(0)
                eproj_group(1, 1)
                scores_group(1)

                # z = zc0 + zc1 on ScalarE (Identity + bias AP) to keep the
                # saturated VectorE free; fast-approx reciprocal is plenty
                # accurate next to bf16 data.
                z = small_pool.tile([128, 1], f32, tag="z")
                rz = small_pool.tile([128, 1], f32, tag="rz")
                with tc.high_priority():
                    nc.scalar.activation(
                        z[:],
                        zc[:, 0:1],
                        mybir.ActivationFunctionType.Identity,
                        bias=zc[:, 1:2],
                    )
                    nc.vector.reciprocal_approx_fast(rz[:], z[:])

                if b == B_LOC - 1:
                    # tail: weights row first on ScalarE, multiplies first on
                    # VectorE, so the copy-accums overlap the remaining STTs.
                    nc.scalar.mul(
                        w_all[0:1, b * N : (b + 1) * N], p_rep[0:1, :], rz[0:1, :]
                    )
                    tail_scr = []
                    for j in (2, 3):
                        scr = small_pool.tile(
                            [128, N], bf16, name=f"scr{j}", tag=f"scr{j}"
                        )
                        nc.vector.tensor_mul(
                            scr[:], enc_all[:, j * N : (j + 1) * N], p_rep[:]
                        )
                        tail_scr.append(scr)
                    for j in (2, 3):
                        sdummy = small_pool.tile(
                            [128, 1], bf16, name="sdummy", tag="sdummy"
                        )
                        nc.scalar.activation(
                            sdummy.broadcast_to((128, N)),
                            tail_scr[j - 2][:],
                            mybir.ActivationFunctionType.Copy,
                            scale=rz[:],
                            accum_out=ctx_all[
                                :, b * E_TILES + j : b * E_TILES + j + 1
                            ],
                        )
                for j in range(E_TILES):
                    if b == B_LOC - 1 and j >= 2:
                        continue
                    dummy = small_pool.tile(
                        [128, 1], bf16, name="dummy", tag="dummy"
                    )
                    nc.vector.scalar_tensor_tensor(
                        out=dummy.broadcast_to((128, N)),
                        in0=enc_all[:, j * N : (j + 1) * N],
                        scalar=rz[:],
                        in1=p_rep[:],
                        op0=mybir.AluOpType.mult,
                        op1=mybir.AluOpType.mult,
                        accum_out=ctx_all[:, b * E_TILES + j : b * E_TILES + j + 1],
                    )

                if b < B_LOC - 1:
                    # weights row: normalize only the replicated row 0 on
                    # ScalarE, off the critical path.
                    nc.scalar.mul(
                        w_all[0:1, b * N : (b + 1) * N], p_rep[0:1, :], rz[0:1, :]
                    )

            # bulk outputs leave while b7 finishes; only the b7 slices remain
            nc.sync.dma_start(
                ctx_out.ap()[:, : 7 * E_TILES], ctx_all[:, : 7 * E_TILES]
            )
            nc.sync.dma_start(w_out.ap()[:, : 7 * N], w_all[:, : 7 * N])
            nc.sync.dma_start(
                ctx_out.ap()[:, 7 * E_TILES :], ctx_all[:, 7 * E_TILES :]
            )
            nc.sync.dma_start(w_out.ap()[:, 7 * N :], w_all[:, 7 * N :])

    nc.compile()
    return nc


def _get_nc():
    if "nc" not in _CACHE:
        _CACHE["nc"] = _build()
    return _CACHE["nc"]


def make_in_maps(hidden, enc_out, attn_w, attn_b, v_w):
    hidden = np.asarray(hidden, dtype=np.float32)
    enc_out = np.asarray(enc_out, dtype=np.float32)
    attn_w = np.asarray(attn_w, dtype=np.float32)
    attn_b = np.asarray(attn_b, dtype=np.float32)
    v_w = np.asarray(v_w, dtype=np.float32)

    Wh = attn_w[:, :DEC_DIM]
    We = attn_w[:, DEC_DIM:]
    h_proj = hidden @ Wh.T + attn_b  # [64, 256]

    wet_np = np.ascontiguousarray(We.T).astype(_BF16)  # [512, 256]
    vrep_np = np.ascontiguousarray(
        np.broadcast_to(v_w[0].reshape(D_TILES, 128, 1), (D_TILES, 128, 128))
    ).astype(_BF16)

    in_maps = []
    for c in range(N_CORES):
        sl = slice(c * B_LOC, (c + 1) * B_LOC)
        encT_np = np.ascontiguousarray(
            enc_out[sl].transpose(0, 2, 1)
        ).astype(_BF16)  # [8, 512, 2048]
        hb_np = np.ascontiguousarray(
            h_proj[sl].reshape(B_LOC, D_TILES, 128).transpose(1, 2, 0)
        )  # [2, 128, 8]
        in_maps.append(
            {"encT": encT_np, "hb": hb_np, "wet": wet_np, "vrep": vrep_np}
        )
    return in_maps


def kernel(hidden, enc_out, attn_w, attn_b, v_w, v_b):
    from concourse.bass_utils import run_bass_kernel_spmd

    nc = _get_nc()
    in_maps = make_in_maps(hidden, enc_out, attn_w, attn_b, v_w)
    res = run_bass_kernel_spmd(nc, in_maps, core_ids=list(range(N_CORES)))

    context = np.concatenate(
        [
            res.results[c]["ctx_out"]
            .reshape(128, B_LOC, E_TILES)
            .transpose(1, 2, 0)
            .reshape(B_LOC, ENC_DIM)
            for c in range(N_CORES)
        ],
        0,
    )
    weights = np.concatenate(
        [
            res.results[c]["w_out"].astype(np.float32).reshape(B_LOC, N)
            for c in range(N_CORES)
        ],
        0,
    )
    return (context, weights)
